# revision 1
# baseline (speedup 1.0000x reference)
"""Trainium2 Bass kernel for the FFF (fast feedforward / MoE-routing) module.

Math (per token x of dim 1024, PAR=8 trees of 255 nodes):
  logits = x @ W_in.T + b_in                      # [B, 2040]
  dec    = logits > 0
  acts   = silu(logits)
  dmap   = indicator of the 8 visited nodes per tree (root + 7 descents,
           descending by dec at the current node)
  out    = (acts * dmap) @ W_out.T                # [B, 1024]

Strategy (8 NeuronCores, data-parallel over the 8192 tokens, 1024 each):
  - GEMM1 in bf16 hi/lo split: 3 passes (hi*hi + hi*lo + lo*hi) for the
    decision-relevant node levels 0..6 (cols 0..1020), 1 pass (hi*hi) for the
    leaf level (cols 1020..2040) whose sign is never used.  PSUM accumulates
    fp32; the fp32 bias is added on the vector engine, so decision signs are
    ~fp32-accurate.
  - dmap is built level-by-level with strided vector ops in a node-major
    column layout (col = 8*node + tree): child1 = V_d * dec_d (stride-2
    upsample), child0 = V_d - child1.
  - masked acts cast to fp16, transposed 128x128 on the PE, GEMM2 in fp16
    (exact products, fp32 PSUM accumulation).
  - weight DMAs are chunked and emitted in need-order so the PE starts
    within a few us instead of waiting for the full 13.6 MB weight load.
"""

import numpy as np
import ml_dtypes

DIM = 1024
PAR = 8
DEPTH = 7
N_NODES = 255
WIDTH = PAR * N_NODES          # 2040
NODES_PAD = 2048               # pad masked-acts/W_out^T to 16*128
N_CORES = 8
TOK_PER_CORE = 1024
TT = 128                       # tokens per tile
NTILES = TOK_PER_CORE // TT    # 8
NT_W = 510                     # GEMM1 n-tile width (4 * 510 = 2040)
K_CH = DIM // 128              # 8 contraction chunks for GEMM1
C_CH = NODES_PAD // 128        # 16 contraction chunks for GEMM2
DEC_COLS = 8 * 127             # 1016: decision nodes are levels 0..6

_PROGRAM = None


def _build_program():
    import concourse.bacc as bacc
    import concourse.tile as tile
    from concourse import mybir
    from concourse.masks import make_identity
    import concourse.bass as bass

    f32 = mybir.dt.float32
    bf16 = mybir.dt.bfloat16
    f16 = mybir.dt.float16
    Alu = mybir.AluOpType
    Act = mybir.ActivationFunctionType

    nc = bacc.Bacc("TRN2", target_bir_lowering=False, debug=False,
                   num_devices=N_CORES)

    # Per-core DRAM I/O (layouts chosen so every DMA has long contiguous
    # runs); xt packs the bf16 hi/lo split as [...,0/1,...]
    xt = nc.dram_tensor("xt", [128, NTILES, 2, K_CH, TT], bf16,
                        kind="ExternalInput")
    w1_hi = nc.dram_tensor("w1_hi", [128, K_CH, WIDTH], bf16,
                           kind="ExternalInput")
    # lo-part only needed for the decision region (cols 0..1020)
    w1_lo = nc.dram_tensor("w1_lo", [128, K_CH, 2 * NT_W], bf16,
                           kind="ExternalInput")
    b1 = nc.dram_tensor("b1", [WIDTH], f32, kind="ExternalInput")
    w2 = nc.dram_tensor("w2", [128, C_CH, DIM], f16, kind="ExternalInput")
    y = nc.dram_tensor("y", [TOK_PER_CORE, DIM], f32, kind="ExternalOutput")

    with tile.TileContext(nc) as tc:
        with (
            tc.tile_pool(name="wts", bufs=1) as wts,
            tc.tile_pool(name="xts", bufs=3) as xts,
            tc.tile_pool(name="logits", bufs=2) as logits_pool,
            tc.tile_pool(name="mask", bufs=2) as mask_pool,
            tc.tile_pool(name="acts", bufs=2) as acts_pool,
            tc.tile_pool(name="out", bufs=2) as out_pool,
            tc.tile_pool(name="pl", bufs=4, space="PSUM") as pl_pool,
            tc.tile_pool(name="pt", bufs=2, space="PSUM") as pt_pool,
            tc.tile_pool(name="py", bufs=2, space="PSUM") as py_pool,
        ):
            # ---- resident weights (DMAs emitted in need-order below) ----
            w1h_sb = wts.tile([128, K_CH, WIDTH], bf16)
            w1l_sb = wts.tile([128, K_CH, 2 * NT_W], bf16)
            w2_sb = wts.tile([128, C_CH, DIM], f16)
            b1_sb = wts.tile([128, WIDTH], f32)
            ident = wts.tile([128, 128], f16)

            xt_tiles = {}

            def prefetch_xt(j, eng=None):
                xhl = xts.tile([128, 2, K_CH, TT], bf16, tag="x")
                (eng or nc.sync).dma_start(out=xhl, in_=xt[:, j, :, :, :])
                xt_tiles[j] = xhl

            # Weight DMAs chunked and emitted on the Sync engine in
            # consumption order (only Sync's HW DGE fans out over all 16
            # queues, ~400 GB/s; each dma_start dispatch costs ~0.6us).
            # x-tile prefetches ride GpSimd's slower SW DGE — their
            # deadlines are generous and this keeps Sync free for weights.
            nc.sync.dma_start(out=w1h_sb[:, 0, :], in_=w1_hi[:, 0, :])
            xhl0 = xts.tile([128, 2, K_CH, TT], bf16, tag="x")
            nc.sync.dma_start(out=xhl0[:, 0], in_=xt[:, 0, 0, :, :])
            nc.sync.dma_start(out=xhl0[:, 1], in_=xt[:, 0, 1, :, :])
            xt_tiles[0] = xhl0
            nc.sync.dma_start(out=w1l_sb[:, 0, :], in_=w1_lo[:, 0, :])
            nc.sync.dma_start(out=w1h_sb[:, 1, :], in_=w1_hi[:, 1, :])
            nc.sync.dma_start(out=w1l_sb[:, 1, :], in_=w1_lo[:, 1, :])
            for k in range(2, K_CH, 2):
                nc.sync.dma_start(out=w1h_sb[:, k:k + 2, :],
                                  in_=w1_hi[:, k:k + 2, :])
                nc.sync.dma_start(out=w1l_sb[:, k:k + 2, :],
                                  in_=w1_lo[:, k:k + 2, :])
            prefetch_xt(1)
            for c in range(0, C_CH, 4):
                nc.sync.dma_start(out=w2_sb[:, c:c + 4, :],
                                  in_=w2[:, c:c + 4, :])
            # bias broadcast rides GpSimd's SW DGE, off the weight path
            b1_bcast = bass.AP(tensor=b1, offset=0, ap=[[0, 128], [1, WIDTH]])
            nc.gpsimd.dma_start(out=b1_sb, in_=b1_bcast)
            make_identity(nc, ident)
            for c in range(C_CH):
                nc.sync.dma_start(out=w2_sb[:, c, :], in_=w2[:, c, :])

            # per-token-tile masked-acts, produced by stage A (GEMM1+mask),
            # consumed by stage B (transpose + GEMM2); 1-deep software
            # pipeline so the PE never waits on the vector-engine epilogue.
            state = {}

            def stage_a(j):
                if j not in xt_tiles:
                    prefetch_xt(j, nc.gpsimd)
                xhl = xt_tiles.pop(j)
                xh, xl = xhl[:, 0], xhl[:, 1]

                lg = logits_pool.tile([TT, WIDTH], f32, tag="lg")
                d1 = mask_pool.tile([TT, DEC_COLS], f16, tag="d1")
                vv = mask_pool.tile([TT, WIDTH], f16, tag="vv")
                ac = acts_pool.tile([TT, WIDTH], f16, tag="ac")
                mk = acts_pool.tile([TT, NODES_PAD], f16, tag="mk")

                for nt in range(4):
                    nsl = slice(nt * NT_W, (nt + 1) * NT_W)
                    pl = pl_pool.tile([TT, NT_W], f32)
                    npass = 3 if nt < 2 else 1
                    nmm = K_CH * npass
                    i = 0
                    for k in range(K_CH):
                        mms = [(xh, w1h_sb)]
                        if npass == 3:
                            mms += [(xh, w1l_sb), (xl, w1h_sb)]
                        for (xx, ww) in mms:
                            nc.tensor.matmul(
                                pl, lhsT=xx[:, k, :], rhs=ww[:, k, nsl],
                                start=(i == 0), stop=(i == nmm - 1))
                            i += 1
                    # bias add (fp32, exact) PSUM -> SBUF
                    nc.vector.tensor_tensor(lg[:, nsl], pl, b1_sb[:, nsl],
                                            Alu.add)
                    if nt == 0:
                        nc.vector.tensor_scalar(
                            d1[:, 0:NT_W], lg[:, 0:NT_W], 0.0, None,
                            Alu.is_gt)
                    elif nt == 1:
                        nc.vector.tensor_scalar(
                            d1[:, NT_W:DEC_COLS], lg[:, NT_W:DEC_COLS],
                            0.0, None, Alu.is_gt)
                    nc.scalar.activation(ac[:, nsl], lg[:, nsl], Act.Silu)

                # tree mask: V_0 = 1 at root cols; then per level
                # child1 = V_d * dec_d, child0 = V_d - child1
                nc.vector.memset(vv[:, 0:8], 1.0)
                for d in range(DEPTH):
                    ld = 8 * (1 << d)
                    c0 = 8 * ((1 << d) - 1)
                    c1 = 8 * ((1 << (d + 1)) - 1)
                    vpar = vv[:, c0:c0 + ld].rearrange("p (i t) -> p i t", t=8)
                    dpar = d1[:, c0:c0 + ld].rearrange("p (i t) -> p i t", t=8)
                    kids = vv[:, c1:c1 + 2 * ld].rearrange(
                        "p (i two t) -> p i two t", two=2, t=8)
                    nc.vector.tensor_tensor(kids[:, :, 1, :], vpar, dpar,
                                            Alu.mult)
                    nc.vector.tensor_tensor(kids[:, :, 0, :], vpar,
                                            kids[:, :, 1, :], Alu.subtract)

                # masked acts (fp16); cols 2040:2048 are zero padding so the
                # last transpose/GEMM2 chunk is a uniform 128 wide
                nc.vector.memset(mk[:, WIDTH:NODES_PAD], 0.0)
                nc.vector.tensor_tensor(mk[:, 0:1024], ac[:, 0:1024],
                                        vv[:, 0:1024], Alu.mult)
                nc.vector.tensor_tensor(mk[:, 1024:WIDTH], ac[:, 1024:WIDTH],
                                        vv[:, 1024:WIDTH], Alu.mult)
                state[j] = mk

            def stage_b(j):
                mk = state.pop(j)
                at = acts_pool.tile([128, C_CH, TT], f16, tag="at")
                # transpose in groups -> one PSUM tile -> one copy; first
                # group is a single chunk so GEMM2 can start immediately
                c = 0
                for gsz in (1, 3, 4, 4, 4):
                    pt = pt_pool.tile([128, 512], f16)
                    for i in range(gsz):
                        nc.tensor.transpose(
                            pt[:, i * 128:(i + 1) * 128],
                            mk[:, (c + i) * 128:(c + i + 1) * 128], ident)
                    nc.scalar.copy(
                        at[:, c:c + gsz, :],
                        pt[:, :gsz * 128].rearrange("p (c t) -> p c t", t=TT))
                    c += gsz
                ys = out_pool.tile([TT, DIM], f32, tag="ys")
                for h in range(2):
                    hs = slice(h * 512, (h + 1) * 512)
                    py = py_pool.tile([TT, 512], f32)
                    for c in range(C_CH):
                        nc.tensor.matmul(
                            py, lhsT=at[:, c, :], rhs=w2_sb[:, c, hs],
                            start=(c == 0), stop=(c == C_CH - 1))
                    nc.vector.tensor_copy(ys[:, hs], py)
                    nc.sync.dma_start(out=y[j * TT:(j + 1) * TT, hs],
                                      in_=ys[:, hs])

            # software pipeline: A(0), A(1), B(0), A(2), B(1), ... B(7)
            stage_a(0)
            for j in range(1, NTILES):
                stage_a(j)
                stage_b(j - 1)
            stage_b(NTILES - 1)

    nc.finalize()
    return nc


def _get_program():
    global _PROGRAM
    if _PROGRAM is None:
        _PROGRAM = _build_program()
    return _PROGRAM


def _split_hi_lo(a):
    hi = a.astype(ml_dtypes.bfloat16)
    lo = (a - hi.astype(np.float32)).astype(ml_dtypes.bfloat16)
    return hi, lo


def kernel(oldx, W_in, b_in, W_out):
    from concourse.bass_utils import run_bass_kernel_spmd

    oldx = np.asarray(oldx)
    W_in = np.asarray(W_in, dtype=np.float32)
    b_in = np.asarray(b_in, dtype=np.float32)
    W_out = np.asarray(W_out, dtype=np.float32)
    x = oldx.reshape(-1, DIM).astype(np.float32)          # [8192, 1024]

    # node-major column permutation: our col 8n+t  <-  ref col 255t+n
    i = np.arange(WIDTH)
    perm = 255 * (i % PAR) + (i // PAR)

    w1t = W_in[perm, :].T.astype(np.float32)              # [1024, 2040]
    w1t_hi, w1t_lo = _split_hi_lo(w1t)
    # [dim, width] -> [128, K_CH, WIDTH] with dim = k*128 + p
    w1_hi = np.ascontiguousarray(
        w1t_hi.reshape(K_CH, 128, WIDTH).transpose(1, 0, 2))
    w1_lo = np.ascontiguousarray(
        w1t_lo.reshape(K_CH, 128, WIDTH).transpose(1, 0, 2)[:, :, :2 * NT_W])
    b1 = np.ascontiguousarray(b_in[perm])

    w2t = np.zeros((NODES_PAD, DIM), np.float32)
    w2t[:WIDTH] = W_out.T[perm, :]
    w2 = np.ascontiguousarray(
        w2t.astype(np.float16).reshape(C_CH, 128, DIM).transpose(1, 0, 2))

    in_maps = []
    for c in range(N_CORES):
        xc = x[c * TOK_PER_CORE:(c + 1) * TOK_PER_CORE]   # [1024, 1024]
        xt_hi, xt_lo = _split_hi_lo(xc.T)                 # [dim, tok]
        # [dim, tok] -> [128, NTILES, K_CH, TT]; dim = k*128+p, tok = j*128+t
        xt_hi = xt_hi.reshape(K_CH, 128, NTILES, TT).transpose(1, 2, 0, 3)
        xt_lo = xt_lo.reshape(K_CH, 128, NTILES, TT).transpose(1, 2, 0, 3)
        xt = np.ascontiguousarray(np.stack([xt_hi, xt_lo], axis=2))
        in_maps.append({
            "xt": xt, "w1_hi": w1_hi, "w1_lo": w1_lo,
            "b1": b1, "w2": w2,
        })

    nc = _get_program()
    res = run_bass_kernel_spmd(nc, in_maps, core_ids=list(range(N_CORES)))
    out = np.concatenate([res.results[c]["y"] for c in range(N_CORES)],
                         axis=0)
    return out.reshape(oldx.shape).astype(np.float32)



# revision 3
# speedup vs baseline: 1.0481x; 1.0481x over previous
"""Trainium2 Bass kernel for the FFF (fast feedforward / MoE-routing) module.

Math (per token x of dim 1024, PAR=8 trees of 255 nodes):
  logits = x @ W_in.T + b_in                      # [B, 2040]
  dec    = logits > 0
  acts   = silu(logits)
  dmap   = indicator of the 8 visited nodes per tree (root + 7 descents,
           descending by dec at the current node)
  out    = (acts * dmap) @ W_out.T                # [B, 1024]

Strategy (8 NeuronCores, data-parallel over the 8192 tokens, 1024 each):
  - GEMM1 in fp16 with region-dependent precision.  Decision flips only
    matter where the tree path visits, and a flip at depth d corrupts
    7-d downstream activations, so shallow nodes get the most passes:
      cols    0..512  (nodes   0..63): x_hi*w + x_lo*w, plus x_hi*w_lo
              on cols 0..256 (nodes 0..31, levels 0..4)   -> ~fp32 signs
      cols  512..1024 (nodes  64..127): x_hi*w + x_lo*w   -> fp16-w signs
      cols 1024..2040 (leaves 128..254): x_hi*w, values only (sign unused)
    The w_lo correction weights are pre-scaled by 2^10 (and x_hi by 2^-10
    on-device) so they stay in fp16 normal range.  fp32 bias is added on
    the vector engine, so near-zero logits keep ~fp32-accurate signs
    where it counts.  Measured on the reference data this gives ~7e-3
    overall rel err (vs 4e-3 for the 2x-more-expensive bf16 3-pass).
  - dmap is built level-by-level with strided vector ops in a node-major
    column layout (col = 8*node + tree): child1 = V_d * dec_d (stride-2
    upsample), child0 = V_d - child1.
  - masked acts cast to fp16, transposed 128x128 on the PE, GEMM2 in fp16
    (exact products, fp32 PSUM accumulation).
  - weight DMAs are emitted in column-slab need-order so the PE starts
    within ~2us and streams GEMM1 while later slabs land.
"""

import numpy as np

DIM = 1024
PAR = 8
DEPTH = 7
N_NODES = 255
WIDTH = PAR * N_NODES          # 2040
NODES_PAD = 2048               # pad masked-acts/W_out^T to 16*128
N_CORES = 8
TOK_PER_CORE = 1024
TT = 128                       # tokens per tile
NTILES = TOK_PER_CORE // TT    # 8
K_CH = DIM // 128              # 8 contraction chunks for GEMM1
C_CH = NODES_PAD // 128        # 16 contraction chunks for GEMM2
DEC_COLS = 8 * 127             # 1016: decision nodes are levels 0..6
SH_COLS = 256                  # w_lo correction region: nodes 0..31
LO_SCALE = 1024.0              # 2^10 keeps w_lo out of fp16 subnormals

_PROGRAM = None


def _build_program():
    import concourse.bacc as bacc
    import concourse.tile as tile
    from concourse import mybir
    from concourse.masks import make_identity
    import concourse.bass as bass

    f32 = mybir.dt.float32
    f16 = mybir.dt.float16
    Alu = mybir.AluOpType
    Act = mybir.ActivationFunctionType

    nc = bacc.Bacc("TRN2", target_bir_lowering=False, debug=False,
                   num_devices=N_CORES)

    # Per-core DRAM I/O (layouts chosen so every DMA has long contiguous
    # runs); xt packs the fp16 hi/lo split as [...,0/1,...]
    xt = nc.dram_tensor("xt", [128, NTILES, 2, K_CH, TT], f16,
                        kind="ExternalInput")
    w1 = nc.dram_tensor("w1", [128, K_CH, WIDTH], f16, kind="ExternalInput")
    # scaled lo-part only needed for the shallow region (cols 0..256)
    w1l = nc.dram_tensor("w1l", [128, K_CH, SH_COLS], f16,
                         kind="ExternalInput")
    b1 = nc.dram_tensor("b1", [WIDTH], f32, kind="ExternalInput")
    w2 = nc.dram_tensor("w2", [128, C_CH, DIM], f16, kind="ExternalInput")
    y = nc.dram_tensor("y", [TOK_PER_CORE, DIM], f32, kind="ExternalOutput")

    with tile.TileContext(nc) as tc:
        with (
            tc.tile_pool(name="wts", bufs=1) as wts,
            tc.tile_pool(name="xts", bufs=3) as xts,
            tc.tile_pool(name="xh2s", bufs=2) as xh2s,
            tc.tile_pool(name="logits", bufs=2) as logits_pool,
            tc.tile_pool(name="mask", bufs=2) as mask_pool,
            tc.tile_pool(name="acts", bufs=2) as acts_pool,
            tc.tile_pool(name="out", bufs=2) as out_pool,
            tc.tile_pool(name="pl", bufs=4, space="PSUM") as pl_pool,
            tc.tile_pool(name="pt", bufs=2, space="PSUM") as pt_pool,
            tc.tile_pool(name="py", bufs=2, space="PSUM") as py_pool,
        ):
            # ---- resident weights (DMAs emitted in need-order below) ----
            w1_sb = wts.tile([128, K_CH, WIDTH], f16)
            w1l_sb = wts.tile([128, K_CH, SH_COLS], f16)
            w2_sb = wts.tile([128, C_CH, DIM], f16)
            b1_sb = wts.tile([128, WIDTH], f32)
            ident = wts.tile([128, 128], f16)

            xt_tiles = {}

            def prefetch_xt(j, eng=None):
                xhl = xts.tile([128, 2, K_CH, TT], f16, tag="x")
                (eng or nc.sync).dma_start(out=xhl, in_=xt[:, j, :, :, :])
                xt_tiles[j] = xhl

            # Weight DMAs in PE consumption order on the Sync engine (only
            # Sync's HW DGE fans out over all 16 queues; each dma_start
            # dispatch costs ~0.6us).  Column-slab order matches the P0 ->
            # P1 -> leaf matmul sequence of tile 0, so the PE starts after
            # ~0.4MB instead of the full 4.7MB weight load.
            xhl0 = xts.tile([128, 2, K_CH, TT], f16, tag="x")
            nc.sync.dma_start(out=xhl0[:, 0], in_=xt[:, 0, 0, :, :])
            xt_tiles[0] = xhl0
            for k in range(K_CH):
                nc.sync.dma_start(out=w1_sb[:, k, 0:512], in_=w1[:, k, 0:512])
                nc.sync.dma_start(out=w1l_sb[:, k, :], in_=w1l[:, k, :])
                if k == 0:
                    nc.sync.dma_start(out=xhl0[:, 1], in_=xt[:, 0, 1, :, :])
            for k in range(K_CH):
                nc.sync.dma_start(out=w1_sb[:, k, 512:1024],
                                  in_=w1[:, k, 512:1024])
            for k in range(K_CH):
                nc.sync.dma_start(out=w1_sb[:, k, 1024:WIDTH],
                                  in_=w1[:, k, 1024:WIDTH])
            prefetch_xt(1)
            for c in range(0, C_CH, 4):
                nc.sync.dma_start(out=w2_sb[:, c:c + 4, :],
                                  in_=w2[:, c:c + 4, :])
            # bias broadcast rides GpSimd's SW DGE, off the weight path
            b1_bcast = bass.AP(tensor=b1, offset=0, ap=[[0, 128], [1, WIDTH]])
            nc.gpsimd.dma_start(out=b1_sb, in_=b1_bcast)
            make_identity(nc, ident)

            # per-token-tile masked-acts, produced by stage A (GEMM1+mask),
            # consumed by stage B (transpose + GEMM2); 1-deep software
            # pipeline so the PE never waits on the vector-engine epilogue.
            state = {}

            def stage_a(j):
                if j not in xt_tiles:
                    prefetch_xt(j, nc.gpsimd)
                xhl = xt_tiles.pop(j)
                xh, xl = xhl[:, 0], xhl[:, 1]
                # scaled hi operand for the w_lo correction pass
                xh2 = xh2s.tile([128, K_CH, TT], f16, tag="xh2")
                nc.vector.tensor_scalar(xh2, xh, 1.0 / LO_SCALE, None,
                                        Alu.mult)

                lg = logits_pool.tile([TT, WIDTH], f32, tag="lg")
                d1 = mask_pool.tile([TT, DEC_COLS], f16, tag="d1")
                vv = mask_pool.tile([TT, WIDTH], f16, tag="vv")
                ac = acts_pool.tile([TT, WIDTH], f16, tag="ac")
                mk = acts_pool.tile([TT, NODES_PAD], f16, tag="mk")

                # P0: cols 0..512, 2-pass + w_lo correction on cols 0..256
                p0 = pl_pool.tile([TT, 512], f32, tag="pl")
                for k in range(K_CH):
                    nc.tensor.matmul(p0, lhsT=xh[:, k, :],
                                     rhs=w1_sb[:, k, 0:512],
                                     start=(k == 0), stop=False)
                    nc.tensor.matmul(p0[:, 0:SH_COLS], lhsT=xh2[:, k, :],
                                     rhs=w1l_sb[:, k, :],
                                     start=False, stop=False)
                    nc.tensor.matmul(p0, lhsT=xl[:, k, :],
                                     rhs=w1_sb[:, k, 0:512],
                                     start=False, stop=(k == K_CH - 1))
                nc.vector.tensor_tensor(lg[:, 0:512], p0, b1_sb[:, 0:512],
                                        Alu.add)
                nc.vector.tensor_scalar(d1[:, 0:512], lg[:, 0:512], 0.0,
                                        None, Alu.is_gt)
                nc.scalar.activation(ac[:, 0:512], lg[:, 0:512], Act.Silu)

                # P1: cols 512..1024, 2-pass (x hi/lo)
                p1 = pl_pool.tile([TT, 512], f32, tag="pl")
                for k in range(K_CH):
                    nc.tensor.matmul(p1, lhsT=xh[:, k, :],
                                     rhs=w1_sb[:, k, 512:1024],
                                     start=(k == 0), stop=False)
                    nc.tensor.matmul(p1, lhsT=xl[:, k, :],
                                     rhs=w1_sb[:, k, 512:1024],
                                     start=False, stop=(k == K_CH - 1))
                nc.vector.tensor_tensor(lg[:, 512:1024], p1,
                                        b1_sb[:, 512:1024], Alu.add)
                nc.vector.tensor_scalar(d1[:, 512:DEC_COLS],
                                        lg[:, 512:DEC_COLS], 0.0, None,
                                        Alu.is_gt)
                nc.scalar.activation(ac[:, 512:1024], lg[:, 512:1024],
                                     Act.Silu)

                # P2/P3: leaf cols 1024..2040, single pass (signs unused)
                for c0, c1 in ((1024, 1532), (1532, WIDTH)):
                    w = c1 - c0
                    p = pl_pool.tile([TT, 512], f32, tag="pl")
                    for k in range(K_CH):
                        nc.tensor.matmul(p[:, 0:w], lhsT=xh[:, k, :],
                                         rhs=w1_sb[:, k, c0:c1],
                                         start=(k == 0),
                                         stop=(k == K_CH - 1))
                    nc.vector.tensor_tensor(lg[:, c0:c1], p[:, 0:w],
                                            b1_sb[:, c0:c1], Alu.add)
                    nc.scalar.activation(ac[:, c0:c1], lg[:, c0:c1],
                                         Act.Silu)

                # tree mask: V_0 = 1 at root cols; then per level
                # child1 = V_d * dec_d, child0 = V_d - child1
                nc.vector.memset(vv[:, 0:8], 1.0)
                for d in range(DEPTH):
                    ld = 8 * (1 << d)
                    c0 = 8 * ((1 << d) - 1)
                    c1 = 8 * ((1 << (d + 1)) - 1)
                    vpar = vv[:, c0:c0 + ld].rearrange("p (i t) -> p i t", t=8)
                    dpar = d1[:, c0:c0 + ld].rearrange("p (i t) -> p i t", t=8)
                    kids = vv[:, c1:c1 + 2 * ld].rearrange(
                        "p (i two t) -> p i two t", two=2, t=8)
                    nc.vector.tensor_tensor(kids[:, :, 1, :], vpar, dpar,
                                            Alu.mult)
                    nc.vector.tensor_tensor(kids[:, :, 0, :], vpar,
                                            kids[:, :, 1, :], Alu.subtract)

                # masked acts (fp16); cols 2040:2048 are zero padding so the
                # last transpose/GEMM2 chunk is a uniform 128 wide
                nc.vector.memset(mk[:, WIDTH:NODES_PAD], 0.0)
                nc.vector.tensor_tensor(mk[:, 0:1024], ac[:, 0:1024],
                                        vv[:, 0:1024], Alu.mult)
                nc.vector.tensor_tensor(mk[:, 1024:WIDTH], ac[:, 1024:WIDTH],
                                        vv[:, 1024:WIDTH], Alu.mult)
                state[j] = mk

            def stage_b(j):
                mk = state.pop(j)
                at = acts_pool.tile([128, C_CH, TT], f16, tag="at")
                # transpose in groups -> one PSUM tile -> one copy; first
                # group is a single chunk so GEMM2 can start immediately
                c = 0
                for gsz in (1, 3, 4, 4, 4):
                    pt = pt_pool.tile([128, 512], f16)
                    for i in range(gsz):
                        nc.tensor.transpose(
                            pt[:, i * 128:(i + 1) * 128],
                            mk[:, (c + i) * 128:(c + i + 1) * 128], ident)
                    nc.scalar.copy(
                        at[:, c:c + gsz, :],
                        pt[:, :gsz * 128].rearrange("p (c t) -> p c t", t=TT))
                    c += gsz
                ys = out_pool.tile([TT, DIM], f32, tag="ys")
                for h in range(2):
                    hs = slice(h * 512, (h + 1) * 512)
                    py = py_pool.tile([TT, 512], f32)
                    for c in range(C_CH):
                        nc.tensor.matmul(
                            py, lhsT=at[:, c, :], rhs=w2_sb[:, c, hs],
                            start=(c == 0), stop=(c == C_CH - 1))
                    nc.vector.tensor_copy(ys[:, hs], py)
                    nc.sync.dma_start(out=y[j * TT:(j + 1) * TT, hs],
                                      in_=ys[:, hs])

            # software pipeline: A(0), A(1), B(0), A(2), B(1), ... B(7)
            stage_a(0)
            for j in range(1, NTILES):
                stage_a(j)
                stage_b(j - 1)
            stage_b(NTILES - 1)

    nc.finalize()
    return nc


def _get_program():
    global _PROGRAM
    if _PROGRAM is None:
        _PROGRAM = _build_program()
    return _PROGRAM


def _split_hi_lo_f16(a):
    hi = a.astype(np.float16)
    lo = (a - hi.astype(np.float32)).astype(np.float16)
    return hi, lo


def kernel(oldx, W_in, b_in, W_out):
    from concourse.bass_utils import run_bass_kernel_spmd

    oldx = np.asarray(oldx)
    W_in = np.asarray(W_in, dtype=np.float32)
    b_in = np.asarray(b_in, dtype=np.float32)
    W_out = np.asarray(W_out, dtype=np.float32)
    x = oldx.reshape(-1, DIM).astype(np.float32)          # [8192, 1024]

    # node-major column permutation: our col 8n+t  <-  ref col 255t+n
    i = np.arange(WIDTH)
    perm = 255 * (i % PAR) + (i // PAR)

    w1t = W_in[perm, :].T.astype(np.float32)              # [1024, 2040]
    w1t_hi = w1t.astype(np.float16)
    w1t_lo = ((w1t - w1t_hi.astype(np.float32)) * LO_SCALE).astype(np.float16)
    # [dim, width] -> [128, K_CH, WIDTH] with dim = k*128 + p
    w1 = np.ascontiguousarray(
        w1t_hi.reshape(K_CH, 128, WIDTH).transpose(1, 0, 2))
    w1l = np.ascontiguousarray(
        w1t_lo.reshape(K_CH, 128, WIDTH).transpose(1, 0, 2)[:, :, :SH_COLS])
    b1 = np.ascontiguousarray(b_in[perm])

    w2t = np.zeros((NODES_PAD, DIM), np.float32)
    w2t[:WIDTH] = W_out.T[perm, :]
    w2 = np.ascontiguousarray(
        w2t.astype(np.float16).reshape(C_CH, 128, DIM).transpose(1, 0, 2))

    in_maps = []
    for c in range(N_CORES):
        xc = x[c * TOK_PER_CORE:(c + 1) * TOK_PER_CORE]   # [1024, 1024]
        xt_hi, xt_lo = _split_hi_lo_f16(xc.T)             # [dim, tok]
        # [dim, tok] -> [128, NTILES, K_CH, TT]; dim = k*128+p, tok = j*128+t
        xt_hi = xt_hi.reshape(K_CH, 128, NTILES, TT).transpose(1, 2, 0, 3)
        xt_lo = xt_lo.reshape(K_CH, 128, NTILES, TT).transpose(1, 2, 0, 3)
        xt = np.ascontiguousarray(np.stack([xt_hi, xt_lo], axis=2))
        in_maps.append({
            "xt": xt, "w1": w1, "w1l": w1l,
            "b1": b1, "w2": w2,
        })

    nc = _get_program()
    res = run_bass_kernel_spmd(nc, in_maps, core_ids=list(range(N_CORES)))
    out = np.concatenate([res.results[c]["y"] for c in range(N_CORES)],
                         axis=0)
    return out.reshape(oldx.shape).astype(np.float32)


# revision 5
# speedup vs baseline: 1.1822x; 1.1279x over previous
"""Trainium2 Bass kernel for the FFF (fast feedforward / MoE-routing) module.

Math (per token x of dim 1024, PAR=8 trees of 255 nodes):
  logits = x @ W_in.T + b_in                      # [B, 2040]
  dec    = logits > 0
  acts   = silu(logits)
  dmap   = indicator of the 8 visited nodes per tree (root + 7 descents,
           descending by dec at the current node)
  out    = (acts * dmap) @ W_out.T                # [B, 1024]

Strategy (8 NeuronCores, data-parallel over the 8192 tokens, 1024 each):
  - GEMM1 in fp16 with region-dependent precision.  A decision flip at
    depth d corrupts 7-d downstream activations, so only the shallow
    nodes (0..31, levels 0..4) get the expensive treatment:
      cols   0..256 (nodes  0..31): x_hi*w + x_lo*w + x_hi*w_lo  (~fp32)
      cols 256..2040 (everything else): single x_hi*w pass
    The w_lo correction weights are pre-scaled by 2^10 (and x_hi by
    2^-10 on-device) so they stay in fp16 normal range.  fp32 bias is
    added on the vector engine.  Measured on the reference data this
    gives ~1.1e-2 overall rel err (gate is 2e-2); the error is a few
    deep-level decision flips on near-tie logits.
  - dmap is built level-by-level with strided vector ops in a node-major
    column layout (col = 8*node + tree): child1 = V_d * dec_d (stride-2
    upsample), child0 = V_d - child1.
  - masked acts cast to fp16, transposed 128x128 on the PE, GEMM2 in fp16
    (exact products, fp32 PSUM accumulation).
  - w1 lives in DRAM as four 512-column slabs so each slab is ONE
    dma_start (dispatches cost ~0.65us on the Sync engine and a single
    transfer already fans out over all 16 DMA queues at ~400GB/s); slabs
    are emitted in PE consumption order so the PE starts at ~10us and
    streams while later slabs land.
"""

import numpy as np

DIM = 1024
PAR = 8
DEPTH = 7
N_NODES = 255
WIDTH = PAR * N_NODES          # 2040
NODES_PAD = 2048               # pad masked-acts/W_out^T to 16*128
N_CORES = 8
TOK_PER_CORE = 1024
TT = 128                       # tokens per tile
NTILES = TOK_PER_CORE // TT    # 8
K_CH = DIM // 128              # 8 contraction chunks for GEMM1
C_CH = NODES_PAD // 128        # 16 contraction chunks for GEMM2
DEC_COLS = 8 * 127             # 1016: decision nodes are levels 0..6
SH_COLS = 256                  # hi/lo-corrected region: nodes 0..31
LO_SCALE = 1024.0              # 2^10 keeps w_lo out of fp16 subnormals
N_SLAB = 4                     # w1 column slabs of 512 (last is 504+pad)

_PROGRAM = None


def _build_program():
    import concourse.bacc as bacc
    import concourse.tile as tile
    from concourse import mybir
    from concourse.masks import make_identity
    import concourse.bass as bass

    f32 = mybir.dt.float32
    f16 = mybir.dt.float16
    Alu = mybir.AluOpType
    Act = mybir.ActivationFunctionType

    nc = bacc.Bacc("TRN2", target_bir_lowering=False, debug=False,
                   num_devices=N_CORES)

    # Per-core DRAM I/O.  w1 is stored as column-slabs so one dma_start
    # moves one slab; xt packs the fp16 hi/lo split as [...,0/1,...]
    xt = nc.dram_tensor("xt", [128, NTILES, 2, K_CH, TT], f16,
                        kind="ExternalInput")
    w1 = nc.dram_tensor("w1", [128, N_SLAB, K_CH, 512], f16,
                        kind="ExternalInput")
    w1l = nc.dram_tensor("w1l", [128, K_CH, SH_COLS], f16,
                         kind="ExternalInput")
    b1 = nc.dram_tensor("b1", [WIDTH], f32, kind="ExternalInput")
    w2 = nc.dram_tensor("w2", [128, C_CH, DIM], f16, kind="ExternalInput")
    y = nc.dram_tensor("y", [TOK_PER_CORE, DIM], f32, kind="ExternalOutput")

    with tile.TileContext(nc) as tc:
        with (
            tc.tile_pool(name="wts", bufs=1) as wts,
            tc.tile_pool(name="xts", bufs=3) as xts,
            tc.tile_pool(name="xh2s", bufs=2) as xh2s,
            tc.tile_pool(name="logits", bufs=2) as logits_pool,
            tc.tile_pool(name="mask", bufs=2) as mask_pool,
            tc.tile_pool(name="acts", bufs=2) as acts_pool,
            tc.tile_pool(name="out", bufs=2) as out_pool,
            tc.tile_pool(name="pl", bufs=4, space="PSUM") as pl_pool,
            tc.tile_pool(name="pt", bufs=2, space="PSUM") as pt_pool,
            tc.tile_pool(name="py", bufs=2, space="PSUM") as py_pool,
        ):
            # ---- resident weights (DMAs emitted in need-order below) ----
            w1_sb = wts.tile([128, N_SLAB, K_CH, 512], f16)
            w1l_sb = wts.tile([128, K_CH, SH_COLS], f16)
            w2_sb = wts.tile([128, C_CH, DIM], f16)
            b1_sb = wts.tile([128, WIDTH], f32)
            ident = wts.tile([128, 128], f16)

            xt_tiles = {}

            def prefetch_xt(j, eng=None):
                xhl = xts.tile([128, 2, K_CH, TT], f16, tag="x")
                (eng or nc.sync).dma_start(out=xhl, in_=xt[:, j, :, :, :])
                xt_tiles[j] = xhl

            # Startup DMAs on the Sync engine in PE consumption order;
            # one dispatch per slab keeps the dispatch cost (~0.65us per
            # dma_start) off the critical path.
            xhl0 = xts.tile([128, 2, K_CH, TT], f16, tag="x")
            nc.sync.dma_start(out=xhl0[:, 0], in_=xt[:, 0, 0, :, :])
            xt_tiles[0] = xhl0
            nc.sync.dma_start(out=w1_sb[:, 0], in_=w1[:, 0])
            nc.sync.dma_start(out=xhl0[:, 1], in_=xt[:, 0, 1, :, :])
            nc.sync.dma_start(out=w1l_sb, in_=w1l[:, :, :])
            for s in range(1, N_SLAB):
                nc.sync.dma_start(out=w1_sb[:, s], in_=w1[:, s])
            prefetch_xt(1)
            nc.sync.dma_start(out=w2_sb[:, 0:8, :], in_=w2[:, 0:8, :])
            nc.sync.dma_start(out=w2_sb[:, 8:16, :], in_=w2[:, 8:16, :])
            # bias broadcast rides GpSimd's SW DGE, off the weight path
            b1_bcast = bass.AP(tensor=b1, offset=0, ap=[[0, 128], [1, WIDTH]])
            nc.gpsimd.dma_start(out=b1_sb, in_=b1_bcast)
            make_identity(nc, ident)

            # per-token-tile masked-acts, produced by stage A (GEMM1+mask),
            # consumed by stage B (transpose + GEMM2); 1-deep software
            # pipeline so the PE never waits on the vector-engine epilogue.
            state = {}

            def stage_a(j):
                if j + 1 < NTILES and j + 1 not in xt_tiles:
                    prefetch_xt(j + 1, nc.gpsimd)
                xhl = xt_tiles.pop(j)
                xh, xl = xhl[:, 0], xhl[:, 1]
                # scaled hi operand for the w_lo correction pass
                xh2 = xh2s.tile([128, K_CH, TT], f16, tag="xh2")
                nc.vector.tensor_scalar(xh2, xh, 1.0 / LO_SCALE, None,
                                        Alu.mult)

                lg = logits_pool.tile([TT, WIDTH], f32, tag="lg")
                d1 = mask_pool.tile([TT, DEC_COLS], f16, tag="d1")
                vv = mask_pool.tile([TT, WIDTH], f16, tag="vv")
                ac = acts_pool.tile([TT, WIDTH], f16, tag="ac")
                mk = acts_pool.tile([TT, NODES_PAD], f16, tag="mk")

                # P0: cols 0..512, 1-pass + x_lo and w_lo correction passes
                # on cols 0..256 (shallow nodes, where flips are costly)
                p0 = pl_pool.tile([TT, 512], f32, tag="pl")
                for k in range(K_CH):
                    nc.tensor.matmul(p0, lhsT=xh[:, k, :],
                                     rhs=w1_sb[:, 0, k, :],
                                     start=(k == 0), stop=False)
                for k in range(K_CH):
                    nc.tensor.matmul(p0[:, 0:SH_COLS], lhsT=xl[:, k, :],
                                     rhs=w1_sb[:, 0, k, 0:SH_COLS],
                                     start=False, stop=False)
                for k in range(K_CH):
                    nc.tensor.matmul(p0[:, 0:SH_COLS], lhsT=xh2[:, k, :],
                                     rhs=w1l_sb[:, k, :],
                                     start=False, stop=(k == K_CH - 1))
                nc.vector.tensor_tensor(lg[:, 0:512], p0, b1_sb[:, 0:512],
                                        Alu.add)
                nc.vector.tensor_scalar(d1[:, 0:512], lg[:, 0:512], 0.0,
                                        None, Alu.is_gt)
                nc.scalar.activation(ac[:, 0:512], lg[:, 0:512], Act.Silu)

                # P1..P3: cols 512..2040, single pass
                for s in range(1, N_SLAB):
                    c0 = 512 * s
                    c1 = min(512 * (s + 1), WIDTH)
                    w = c1 - c0
                    p = pl_pool.tile([TT, 512], f32, tag="pl")
                    for k in range(K_CH):
                        nc.tensor.matmul(p[:, 0:w], lhsT=xh[:, k, :],
                                         rhs=w1_sb[:, s, k, 0:w],
                                         start=(k == 0),
                                         stop=(k == K_CH - 1))
                    nc.vector.tensor_tensor(lg[:, c0:c1], p[:, 0:w],
                                            b1_sb[:, c0:c1], Alu.add)
                    if s == 1:
                        nc.vector.tensor_scalar(d1[:, 512:DEC_COLS],
                                                lg[:, 512:DEC_COLS], 0.0,
                                                None, Alu.is_gt)
                    nc.scalar.activation(ac[:, c0:c1], lg[:, c0:c1],
                                         Act.Silu)

                # tree mask: V_0 = 1 at root cols; then per level
                # child1 = V_d * dec_d, child0 = V_d - child1
                nc.vector.memset(vv[:, 0:8], 1.0)
                for d in range(DEPTH):
                    ld = 8 * (1 << d)
                    c0 = 8 * ((1 << d) - 1)
                    c1 = 8 * ((1 << (d + 1)) - 1)
                    vpar = vv[:, c0:c0 + ld].rearrange("p (i t) -> p i t", t=8)
                    dpar = d1[:, c0:c0 + ld].rearrange("p (i t) -> p i t", t=8)
                    kids = vv[:, c1:c1 + 2 * ld].rearrange(
                        "p (i two t) -> p i two t", two=2, t=8)
                    nc.vector.tensor_tensor(kids[:, :, 1, :], vpar, dpar,
                                            Alu.mult)
                    nc.vector.tensor_tensor(kids[:, :, 0, :], vpar,
                                            kids[:, :, 1, :], Alu.subtract)

                # masked acts (fp16); cols 2040:2048 are zero padding so the
                # last transpose/GEMM2 chunk is a uniform 128 wide
                nc.vector.memset(mk[:, WIDTH:NODES_PAD], 0.0)
                nc.vector.tensor_tensor(mk[:, 0:1024], ac[:, 0:1024],
                                        vv[:, 0:1024], Alu.mult)
                nc.vector.tensor_tensor(mk[:, 1024:WIDTH], ac[:, 1024:WIDTH],
                                        vv[:, 1024:WIDTH], Alu.mult)
                state[j] = mk

            def stage_b(j):
                mk = state.pop(j)
                at = acts_pool.tile([128, C_CH, TT], f16, tag="at")
                # transpose in groups -> one PSUM tile -> one copy; first
                # group is a single chunk so GEMM2 can start immediately
                c = 0
                for gsz in (1, 3, 4, 4, 4):
                    pt = pt_pool.tile([128, 512], f16)
                    for i in range(gsz):
                        nc.tensor.transpose(
                            pt[:, i * 128:(i + 1) * 128],
                            mk[:, (c + i) * 128:(c + i + 1) * 128], ident)
                    nc.scalar.copy(
                        at[:, c:c + gsz, :],
                        pt[:, :gsz * 128].rearrange("p (c t) -> p c t", t=TT))
                    c += gsz
                ys = out_pool.tile([TT, DIM], f32, tag="ys")
                for h in range(2):
                    hs = slice(h * 512, (h + 1) * 512)
                    py = py_pool.tile([TT, 512], f32)
                    for c in range(C_CH):
                        nc.tensor.matmul(
                            py, lhsT=at[:, c, :], rhs=w2_sb[:, c, hs],
                            start=(c == 0), stop=(c == C_CH - 1))
                    nc.vector.tensor_copy(ys[:, hs], py)
                    nc.sync.dma_start(out=y[j * TT:(j + 1) * TT, hs],
                                      in_=ys[:, hs])

            # software pipeline: A(0), A(1), B(0), A(2), B(1), ... B(7)
            stage_a(0)
            for j in range(1, NTILES):
                stage_a(j)
                stage_b(j - 1)
            stage_b(NTILES - 1)

    nc.finalize()
    return nc


def _get_program():
    global _PROGRAM
    if _PROGRAM is None:
        _PROGRAM = _build_program()
    return _PROGRAM


def _split_hi_lo_f16(a):
    hi = a.astype(np.float16)
    lo = (a - hi.astype(np.float32)).astype(np.float16)
    return hi, lo


def kernel(oldx, W_in, b_in, W_out):
    from concourse.bass_utils import run_bass_kernel_spmd

    oldx = np.asarray(oldx)
    W_in = np.asarray(W_in, dtype=np.float32)
    b_in = np.asarray(b_in, dtype=np.float32)
    W_out = np.asarray(W_out, dtype=np.float32)
    x = oldx.reshape(-1, DIM).astype(np.float32)          # [8192, 1024]

    # node-major column permutation: our col 8n+t  <-  ref col 255t+n
    i = np.arange(WIDTH)
    perm = 255 * (i % PAR) + (i // PAR)

    w1t = W_in[perm, :].T.astype(np.float32)              # [1024, 2040]
    w1t_hi = w1t.astype(np.float16)
    w1t_lo = ((w1t - w1t_hi.astype(np.float32)) * LO_SCALE).astype(np.float16)
    # [dim, width] -> [128, N_SLAB, K_CH, 512] with dim = k*128 + p,
    # width col = 512*slab + c (last slab zero-padded to 512)
    w1p = np.zeros((1024, N_SLAB * 512), np.float16)
    w1p[:, :WIDTH] = w1t_hi
    w1 = np.ascontiguousarray(
        w1p.reshape(K_CH, 128, N_SLAB, 512).transpose(1, 2, 0, 3))
    w1l = np.ascontiguousarray(
        w1t_lo.reshape(K_CH, 128, WIDTH).transpose(1, 0, 2)[:, :, :SH_COLS])
    b1 = np.ascontiguousarray(b_in[perm])

    w2t = np.zeros((NODES_PAD, DIM), np.float32)
    w2t[:WIDTH] = W_out.T[perm, :]
    w2 = np.ascontiguousarray(
        w2t.astype(np.float16).reshape(C_CH, 128, DIM).transpose(1, 0, 2))

    in_maps = []
    for c in range(N_CORES):
        xc = x[c * TOK_PER_CORE:(c + 1) * TOK_PER_CORE]   # [1024, 1024]
        xt_hi, xt_lo = _split_hi_lo_f16(xc.T)             # [dim, tok]
        # [dim, tok] -> [128, NTILES, K_CH, TT]; dim = k*128+p, tok = j*128+t
        xt_hi = xt_hi.reshape(K_CH, 128, NTILES, TT).transpose(1, 2, 0, 3)
        xt_lo = xt_lo.reshape(K_CH, 128, NTILES, TT).transpose(1, 2, 0, 3)
        xt = np.ascontiguousarray(np.stack([xt_hi, xt_lo], axis=2))
        in_maps.append({
            "xt": xt, "w1": w1, "w1l": w1l,
            "b1": b1, "w2": w2,
        })

    nc = _get_program()
    res = run_bass_kernel_spmd(nc, in_maps, core_ids=list(range(N_CORES)))
    out = np.concatenate([res.results[c]["y"] for c in range(N_CORES)],
                         axis=0)
    return out.reshape(oldx.shape).astype(np.float32)


# revision 9
# speedup vs baseline: 1.2601x; 1.0659x over previous
"""Trainium2 Bass kernel for the FFF (fast feedforward / MoE-routing) module.

Math (per token x of dim 1024, PAR=8 trees of 255 nodes):
  logits = x @ W_in.T + b_in                      # [B, 2040]
  dec    = logits > 0
  acts   = silu(logits)
  dmap   = indicator of the 8 visited nodes per tree (root + 7 descents,
           descending by dec at the current node)
  out    = (acts * dmap) @ W_out.T                # [B, 1024]

Strategy (8 NeuronCores, data-parallel over the 8192 tokens, 1024 each):
  - GEMM1 in fp16 with region-dependent precision.  A decision flip at
    depth d corrupts 7-d downstream activations, so only the shallow
    nodes (0..31, levels 0..4) get the expensive treatment:
      cols   0..256 (nodes  0..31): x_hi*w + x_lo*w + x_hi*w_lo  (~fp32)
      cols 256..2040 (everything else): single x_hi*w pass
    The w_lo correction weights are pre-scaled by 2^10 (and x_hi by
    2^-10 on-device) so they stay in fp16 normal range.  fp32 bias is
    added on the vector engine.  Measured on the reference data this
    gives ~1.1e-2 overall rel err (gate is 2e-2).
  - dmap is built level-by-level with strided vector ops in a node-major
    column layout (col = 8*node + tree): child1 = V_d * dec_d (stride-2
    upsample), child0 = V_d - child1.
  - masked acts cast to fp16, transposed 128x128 on the PE, GEMM2 in fp16
    (exact products, fp32 PSUM accumulation).
  - startup: the 4.7MB of GEMM1 weights stream in as four 512-column
    slabs (one dma_start each; the DGE fair-shares ~300GB/s across
    in-flight dispatches, so fewer+ordered dispatches = earliest slab0).
    Tiles 0 and 1 are processed SLAB-MAJOR so each arriving slab feeds
    2 tiles of PE work; the fp32 bias is broadcast across partitions on
    the PE (K=1 matmul vs ones) during the initial weight wait instead
    of a 1MB broadcast DMA competing for early bandwidth.
"""

import numpy as np

DIM = 1024
PAR = 8
DEPTH = 7
N_NODES = 255
WIDTH = PAR * N_NODES          # 2040
NODES_PAD = 2048               # pad masked-acts/W_out^T to 16*128
N_CORES = 8
TOK_PER_CORE = 1024
TT = 128                       # tokens per tile
NTILES = TOK_PER_CORE // TT    # 8
K_CH = DIM // 128              # 8 contraction chunks for GEMM1
C_CH = NODES_PAD // 128        # 16 contraction chunks for GEMM2
DEC_COLS = 8 * 127             # 1016: decision nodes are levels 0..6
SH_COLS = 256                  # hi/lo-corrected region: nodes 0..31
LO_SCALE = 1024.0              # 2^10 keeps w_lo out of fp16 subnormals
N_SLAB = 4                     # w1 column slabs of 512 (last is 504+pad)

_PROGRAM = None


def _build_program():
    import concourse.bacc as bacc
    import concourse.tile as tile
    from concourse import mybir
    from concourse.masks import make_identity
    import concourse.bass as bass

    f32 = mybir.dt.float32
    f16 = mybir.dt.float16
    Alu = mybir.AluOpType
    Act = mybir.ActivationFunctionType

    nc = bacc.Bacc("TRN2", target_bir_lowering=False, debug=False,
                   num_devices=N_CORES)

    xt = nc.dram_tensor("xt", [128, NTILES, 2, K_CH, TT], f16,
                        kind="ExternalInput")
    w1 = nc.dram_tensor("w1", [128, N_SLAB, K_CH, 512], f16,
                        kind="ExternalInput")
    w1l = nc.dram_tensor("w1l", [128, K_CH, SH_COLS], f16,
                         kind="ExternalInput")
    b1 = nc.dram_tensor("b1", [WIDTH], f32, kind="ExternalInput")
    w2 = nc.dram_tensor("w2", [128, C_CH, DIM], f16, kind="ExternalInput")
    y = nc.dram_tensor("y", [TOK_PER_CORE, DIM], f32, kind="ExternalOutput")

    SLAB_LIM = [(0, 512), (512, 1024), (1024, 1536), (1536, WIDTH)]

    with tile.TileContext(nc) as tc:
        with (
            tc.tile_pool(name="wts", bufs=1) as wts,
            tc.tile_pool(name="xts", bufs=4) as xts,
            tc.tile_pool(name="xh2s", bufs=2) as xh2s,
            tc.tile_pool(name="logits", bufs=2) as logits_pool,
            tc.tile_pool(name="mask", bufs=2) as mask_pool,
            tc.tile_pool(name="acts", bufs=2) as acts_pool,
            tc.tile_pool(name="mks", bufs=3) as mks_pool,
            tc.tile_pool(name="out", bufs=2) as out_pool,
            tc.tile_pool(name="pl", bufs=4, space="PSUM") as pl_pool,
            tc.tile_pool(name="pt", bufs=2, space="PSUM") as pt_pool,
            tc.tile_pool(name="py", bufs=2, space="PSUM") as py_pool,
        ):
            # ---- resident weights ----
            w1_sb = wts.tile([128, N_SLAB, K_CH, 512], f16)
            w1l_sb = wts.tile([128, K_CH, SH_COLS], f16)
            w2_sb = wts.tile([128, C_CH, DIM], f16)
            b1_sb = wts.tile([128, WIDTH], f32)
            b1_row = wts.tile([1, WIDTH], f32)
            ones = wts.tile([1, 128], f32)
            ident = wts.tile([128, 128], f16)

            xt_tiles = {}

            def prefetch_xt(j):
                xhl = xts.tile([128, 2, K_CH, TT], f16, tag="x")
                nc.sync.dma_start(out=xhl, in_=xt[:, j, :, :, :])
                xt_tiles[j] = xhl

            # Startup DMAs on the Sync engine in PE consumption order.
            # The DGE fair-shares bandwidth over in-flight dispatches and
            # completes them in dispatch order, so this order == arrival
            # order.
            b1_row_src = bass.AP(tensor=b1, offset=0, ap=[[0, 1], [1, WIDTH]])
            nc.sync.dma_start(out=b1_row, in_=b1_row_src)
            xhl0 = xts.tile([128, 2, K_CH, TT], f16, tag="x")
            xhl1 = xts.tile([128, 2, K_CH, TT], f16, tag="x")
            nc.sync.dma_start(out=xhl0[:, 0], in_=xt[:, 0, 0, :, :])
            nc.sync.dma_start(out=w1_sb[:, 0], in_=w1[:, 0])
            nc.sync.dma_start(out=xhl0[:, 1], in_=xt[:, 0, 1, :, :])
            nc.sync.dma_start(out=w1l_sb, in_=w1l[:, :, :])
            nc.sync.dma_start(out=xhl1[:, 0], in_=xt[:, 1, 0, :, :])
            nc.sync.dma_start(out=xhl1[:, 1], in_=xt[:, 1, 1, :, :])
            xt_tiles[0] = xhl0
            xt_tiles[1] = xhl1
            for s in range(1, N_SLAB):
                nc.sync.dma_start(out=w1_sb[:, s], in_=w1[:, s])
            prefetch_xt(2)
            nc.sync.dma_start(out=w2_sb[:, 0:8, :], in_=w2[:, 0:8, :])
            prefetch_xt(3)
            nc.sync.dma_start(out=w2_sb[:, 8:16, :], in_=w2[:, 8:16, :])
            make_identity(nc, ident)
            nc.vector.memset(ones, 1.0)

            # fp32 bias broadcast across partitions on the PE (exact:
            # 1.0 * b accumulated in fp32) while weights stream in.
            for c0, c1 in SLAB_LIM:
                pb = pl_pool.tile([TT, 512], f32, tag="pl")
                nc.tensor.matmul(pb[:, 0:c1 - c0], lhsT=ones,
                                 rhs=b1_row[:, c0:c1], start=True, stop=True)
                nc.vector.tensor_copy(b1_sb[:, c0:c1], pb[:, 0:c1 - c0])

            state = {}

            def epilogue_vec(j, lg, d1, vv, ac):
                # tree mask: V_0 = 1 at root cols; then per level
                # child1 = V_d * dec_d, child0 = V_d - child1
                nc.vector.memset(vv[:, 0:8], 1.0)
                for d in range(DEPTH):
                    ld = 8 * (1 << d)
                    c0 = 8 * ((1 << d) - 1)
                    c1 = 8 * ((1 << (d + 1)) - 1)
                    vpar = vv[:, c0:c0 + ld].rearrange("p (i t) -> p i t", t=8)
                    dpar = d1[:, c0:c0 + ld].rearrange("p (i t) -> p i t", t=8)
                    kids = vv[:, c1:c1 + 2 * ld].rearrange(
                        "p (i two t) -> p i two t", two=2, t=8)
                    nc.vector.tensor_tensor(kids[:, :, 1, :], vpar, dpar,
                                            Alu.mult)
                    nc.vector.tensor_tensor(kids[:, :, 0, :], vpar,
                                            kids[:, :, 1, :], Alu.subtract)

            def finish_mask(j, ac, vv):
                mk = mks_pool.tile([TT, NODES_PAD], f16, tag="mk")
                nc.vector.memset(mk[:, WIDTH:NODES_PAD], 0.0)
                nc.vector.tensor_tensor(mk[:, 0:1024], ac[:, 0:1024],
                                        vv[:, 0:1024], Alu.mult)
                nc.vector.tensor_tensor(mk[:, 1024:WIDTH], ac[:, 1024:WIDTH],
                                        vv[:, 1024:WIDTH], Alu.mult)
                state[j] = mk

            def gemm1_slab(s, xh, xl, xh2, lg, d1, ac):
                """Matmuls + per-slab epilogue for slab s of one tile."""
                c0, c1 = SLAB_LIM[s]
                w = c1 - c0
                p = pl_pool.tile([TT, 512], f32, tag="pl")
                if s == 0:
                    for k in range(K_CH):
                        nc.tensor.matmul(p, lhsT=xh[:, k, :],
                                         rhs=w1_sb[:, 0, k, :],
                                         start=(k == 0), stop=False)
                    for k in range(K_CH):
                        nc.tensor.matmul(p[:, 0:SH_COLS], lhsT=xl[:, k, :],
                                         rhs=w1_sb[:, 0, k, 0:SH_COLS],
                                         start=False, stop=False)
                    for k in range(K_CH):
                        nc.tensor.matmul(p[:, 0:SH_COLS], lhsT=xh2[:, k, :],
                                         rhs=w1l_sb[:, k, :],
                                         start=False, stop=(k == K_CH - 1))
                else:
                    for k in range(K_CH):
                        nc.tensor.matmul(p[:, 0:w], lhsT=xh[:, k, :],
                                         rhs=w1_sb[:, s, k, 0:w],
                                         start=(k == 0),
                                         stop=(k == K_CH - 1))
                nc.vector.tensor_tensor(lg[:, c0:c1], p[:, 0:w],
                                        b1_sb[:, c0:c1], Alu.add)
                if s == 0:
                    nc.vector.tensor_scalar(d1[:, 0:512], lg[:, 0:512], 0.0,
                                            None, Alu.is_gt)
                elif s == 1:
                    nc.vector.tensor_scalar(d1[:, 512:DEC_COLS],
                                            lg[:, 512:DEC_COLS], 0.0,
                                            None, Alu.is_gt)
                nc.scalar.activation(ac[:, c0:c1], lg[:, c0:c1], Act.Silu)

            def tile_bufs(j):
                xhl = xt_tiles.pop(j)
                xh, xl = xhl[:, 0], xhl[:, 1]
                xh2 = xh2s.tile([128, K_CH, TT], f16, tag="xh2")
                nc.vector.tensor_scalar(xh2, xh, 1.0 / LO_SCALE, None,
                                        Alu.mult)
                lg = logits_pool.tile([TT, WIDTH], f32, tag="lg")
                d1 = mask_pool.tile([TT, DEC_COLS], f16, tag="d1")
                vv = mask_pool.tile([TT, WIDTH], f16, tag="vv")
                ac = acts_pool.tile([TT, WIDTH], f16, tag="ac")
                return xh, xl, xh2, lg, d1, vv, ac

            def stage_a(j):
                if j + 1 < NTILES and j + 1 not in xt_tiles:
                    prefetch_xt(j + 1)
                xh, xl, xh2, lg, d1, vv, ac = tile_bufs(j)
                for s in range(N_SLAB):
                    gemm1_slab(s, xh, xl, xh2, lg, d1, ac)
                epilogue_vec(j, lg, d1, vv, ac)
                finish_mask(j, ac, vv)

            def stage_ab01():
                # tiles 0 and 1 slab-major: each arriving w1 slab feeds
                # 2 tiles of PE work, halving the DMA-bound startup.
                b0 = tile_bufs(0)
                b1_ = tile_bufs(1)
                for s in range(N_SLAB):
                    gemm1_slab(s, b0[0], b0[1], b0[2], b0[3], b0[4], b0[6])
                    gemm1_slab(s, b1_[0], b1_[1], b1_[2], b1_[3], b1_[4],
                               b1_[6])
                    if s == 1:
                        epilogue_vec(0, b0[3], b0[4], b0[5], b0[6])
                        epilogue_vec(1, b1_[3], b1_[4], b1_[5], b1_[6])
                finish_mask(0, b0[6], b0[5])
                finish_mask(1, b1_[6], b1_[5])

            def stage_b(j):
                mk = state.pop(j)
                at = acts_pool.tile([128, C_CH, TT], f16, tag="at")
                c = 0
                for gsz in (1, 3, 4, 4, 4):
                    pt = pt_pool.tile([128, 512], f16)
                    for i in range(gsz):
                        nc.tensor.transpose(
                            pt[:, i * 128:(i + 1) * 128],
                            mk[:, (c + i) * 128:(c + i + 1) * 128], ident)
                    nc.scalar.copy(
                        at[:, c:c + gsz, :],
                        pt[:, :gsz * 128].rearrange("p (c t) -> p c t", t=TT))
                    c += gsz
                ys = out_pool.tile([TT, DIM], f32, tag="ys")
                # c-outer so w2 chunks are consumed in arrival order
                py0 = py_pool.tile([TT, 512], f32, tag="py")
                py1 = py_pool.tile([TT, 512], f32, tag="py")
                for c in range(C_CH):
                    nc.tensor.matmul(py0, lhsT=at[:, c, :],
                                     rhs=w2_sb[:, c, 0:512],
                                     start=(c == 0), stop=(c == C_CH - 1))
                    nc.tensor.matmul(py1, lhsT=at[:, c, :],
                                     rhs=w2_sb[:, c, 512:1024],
                                     start=(c == 0), stop=(c == C_CH - 1))
                for h, py in ((0, py0), (1, py1)):
                    hs = slice(h * 512, (h + 1) * 512)
                    nc.vector.tensor_copy(ys[:, hs], py)
                    nc.sync.dma_start(out=y[j * TT:(j + 1) * TT, hs],
                                      in_=ys[:, hs])

            # pipeline: AB(0,1), A(2), B(0), A(3), B(1), ... A(7), B(5),
            # B(6), B(7)
            stage_ab01()
            for j in range(2, NTILES):
                stage_a(j)
                stage_b(j - 2)
            stage_b(NTILES - 2)
            stage_b(NTILES - 1)

    nc.finalize()
    return nc


def _get_program():
    global _PROGRAM
    if _PROGRAM is None:
        _PROGRAM = _build_program()
    return _PROGRAM


def _split_hi_lo_f16(a):
    hi = a.astype(np.float16)
    lo = (a - hi.astype(np.float32)).astype(np.float16)
    return hi, lo


def kernel(oldx, W_in, b_in, W_out):
    from concourse.bass_utils import run_bass_kernel_spmd

    oldx = np.asarray(oldx)
    W_in = np.asarray(W_in, dtype=np.float32)
    b_in = np.asarray(b_in, dtype=np.float32)
    W_out = np.asarray(W_out, dtype=np.float32)
    x = oldx.reshape(-1, DIM).astype(np.float32)          # [8192, 1024]

    # node-major column permutation: our col 8n+t  <-  ref col 255t+n
    i = np.arange(WIDTH)
    perm = 255 * (i % PAR) + (i // PAR)

    w1t = W_in[perm, :].T.astype(np.float32)              # [1024, 2040]
    w1t_hi = w1t.astype(np.float16)
    w1t_lo = ((w1t - w1t_hi.astype(np.float32)) * LO_SCALE).astype(np.float16)
    # [dim, width] -> [128, N_SLAB, K_CH, 512] with dim = k*128 + p,
    # width col = 512*slab + c (last slab zero-padded to 512)
    w1p = np.zeros((1024, N_SLAB * 512), np.float16)
    w1p[:, :WIDTH] = w1t_hi
    w1 = np.ascontiguousarray(
        w1p.reshape(K_CH, 128, N_SLAB, 512).transpose(1, 2, 0, 3))
    w1l = np.ascontiguousarray(
        w1t_lo.reshape(K_CH, 128, WIDTH).transpose(1, 0, 2)[:, :, :SH_COLS])
    b1 = np.ascontiguousarray(b_in[perm])

    w2t = np.zeros((NODES_PAD, DIM), np.float32)
    w2t[:WIDTH] = W_out.T[perm, :]
    w2 = np.ascontiguousarray(
        w2t.astype(np.float16).reshape(C_CH, 128, DIM).transpose(1, 0, 2))

    in_maps = []
    for c in range(N_CORES):
        xc = x[c * TOK_PER_CORE:(c + 1) * TOK_PER_CORE]   # [1024, 1024]
        xt_hi, xt_lo = _split_hi_lo_f16(xc.T)             # [dim, tok]
        # [dim, tok] -> [128, NTILES, K_CH, TT]; dim = k*128+p, tok = j*128+t
        xt_hi = xt_hi.reshape(K_CH, 128, NTILES, TT).transpose(1, 2, 0, 3)
        xt_lo = xt_lo.reshape(K_CH, 128, NTILES, TT).transpose(1, 2, 0, 3)
        xt = np.ascontiguousarray(np.stack([xt_hi, xt_lo], axis=2))
        in_maps.append({
            "xt": xt, "w1": w1, "w1l": w1l,
            "b1": b1, "w2": w2,
        })

    nc = _get_program()
    res = run_bass_kernel_spmd(nc, in_maps, core_ids=list(range(N_CORES)))
    out = np.concatenate([res.results[c]["y"] for c in range(N_CORES)],
                         axis=0)
    return out.reshape(oldx.shape).astype(np.float32)


# revision 10
# speedup vs baseline: 1.3158x; 1.0442x over previous
"""Trainium2 Bass kernel for the FFF (fast feedforward / MoE-routing) module.

Math (per token x of dim 1024, PAR=8 trees of 255 nodes):
  logits = x @ W_in.T + b_in                      # [B, 2040]
  dec    = logits > 0
  acts   = silu(logits)
  dmap   = indicator of the 8 visited nodes per tree (root + 7 descents,
           descending by dec at the current node)
  out    = (acts * dmap) @ W_out.T                # [B, 1024]

Strategy (8 NeuronCores, data-parallel over the 8192 tokens, 1024 each):
  - GEMM1 in fp16 with region-dependent precision.  A decision flip at
    depth d corrupts 7-d downstream activations, so only the shallow
    nodes (0..15, levels 0..3) get the expensive treatment:
      cols   0..128 (nodes  0..15): x_hi*w + x_lo*w + x_hi*w_lo  (~fp32)
      cols 128..2040 (everything else): single x_hi*w pass
    The w_lo correction weights are pre-scaled by 2^10 (and x_hi by
    2^-10 on-device) so they stay in fp16 normal range.  fp32 bias is
    added on the vector engine.  Measured on the reference data this
    gives ~1.35e-2 overall rel err (gate is 2e-2).
  - dmap is built level-by-level with strided vector ops in a node-major
    column layout (col = 8*node + tree): child1 = V_d * dec_d (stride-2
    upsample), child0 = V_d - child1.
  - masked acts cast to fp16, transposed 128x128 on the PE, GEMM2 in fp16
    (exact products, fp32 PSUM accumulation).
  - startup: the 4.7MB of GEMM1 weights stream in as four 512-column
    slabs (one dma_start each; the DGE fair-shares ~300GB/s across
    in-flight dispatches, so fewer+ordered dispatches = earliest slab0).
    Tiles 0 and 1 are processed SLAB-MAJOR so each arriving slab feeds
    2 tiles of PE work; the fp32 bias is broadcast across partitions on
    the PE (K=1 matmul vs ones) during the initial weight wait instead
    of a 1MB broadcast DMA competing for early bandwidth.
"""

import numpy as np

DIM = 1024
PAR = 8
DEPTH = 7
N_NODES = 255
WIDTH = PAR * N_NODES          # 2040
NODES_PAD = 2048               # pad masked-acts/W_out^T to 16*128
N_CORES = 8
TOK_PER_CORE = 1024
TT = 128                       # tokens per tile
NTILES = TOK_PER_CORE // TT    # 8
K_CH = DIM // 128              # 8 contraction chunks for GEMM1
C_CH = NODES_PAD // 128        # 16 contraction chunks for GEMM2
DEC_COLS = 8 * 127             # 1016: decision nodes are levels 0..6
SH_COLS = 128                  # hi/lo-corrected region: nodes 0..15
LO_SCALE = 1024.0              # 2^10 keeps w_lo out of fp16 subnormals
N_SLAB = 4                     # w1 column slabs of 512 (last is 504+pad)

_PROGRAM = None


def _build_program():
    import concourse.bacc as bacc
    import concourse.tile as tile
    from concourse import mybir
    from concourse.masks import make_identity
    import concourse.bass as bass

    f32 = mybir.dt.float32
    f16 = mybir.dt.float16
    Alu = mybir.AluOpType
    Act = mybir.ActivationFunctionType

    nc = bacc.Bacc("TRN2", target_bir_lowering=False, debug=False,
                   num_devices=N_CORES)

    xt = nc.dram_tensor("xt", [128, NTILES, 2, K_CH, TT], f16,
                        kind="ExternalInput")
    w1 = nc.dram_tensor("w1", [128, N_SLAB, K_CH, 512], f16,
                        kind="ExternalInput")
    w1l = nc.dram_tensor("w1l", [128, K_CH, SH_COLS], f16,
                         kind="ExternalInput")
    b1 = nc.dram_tensor("b1", [WIDTH], f32, kind="ExternalInput")
    w2 = nc.dram_tensor("w2", [128, C_CH, DIM], f16, kind="ExternalInput")
    y = nc.dram_tensor("y", [TOK_PER_CORE, DIM], f32, kind="ExternalOutput")

    SLAB_LIM = [(0, 512), (512, 1024), (1024, 1536), (1536, WIDTH)]

    with tile.TileContext(nc) as tc:
        with (
            tc.tile_pool(name="wts", bufs=1) as wts,
            tc.tile_pool(name="xts", bufs=4) as xts,
            tc.tile_pool(name="xh2s", bufs=2) as xh2s,
            tc.tile_pool(name="logits", bufs=2) as logits_pool,
            tc.tile_pool(name="mask", bufs=2) as mask_pool,
            tc.tile_pool(name="acts", bufs=2) as acts_pool,
            tc.tile_pool(name="mks", bufs=3) as mks_pool,
            tc.tile_pool(name="out", bufs=2) as out_pool,
            tc.tile_pool(name="pl", bufs=4, space="PSUM") as pl_pool,
            tc.tile_pool(name="pt", bufs=2, space="PSUM") as pt_pool,
            tc.tile_pool(name="py", bufs=2, space="PSUM") as py_pool,
        ):
            # ---- resident weights ----
            w1_sb = wts.tile([128, N_SLAB, K_CH, 512], f16)
            w1l_sb = wts.tile([128, K_CH, SH_COLS], f16)
            w2_sb = wts.tile([128, C_CH, DIM], f16)
            b1_sb = wts.tile([128, WIDTH], f32)
            b1_row = wts.tile([1, WIDTH], f32)
            ones = wts.tile([1, 128], f32)
            ident = wts.tile([128, 128], f16)

            xt_tiles = {}

            def prefetch_xt(j):
                xhl = xts.tile([128, 2, K_CH, TT], f16, tag="x")
                nc.sync.dma_start(out=xhl, in_=xt[:, j, :, :, :])
                xt_tiles[j] = xhl

            # Startup DMAs on the Sync engine in PE consumption order.
            # The DGE fair-shares bandwidth over in-flight dispatches and
            # completes them in dispatch order, so this order == arrival
            # order.
            b1_row_src = bass.AP(tensor=b1, offset=0, ap=[[0, 1], [1, WIDTH]])
            nc.sync.dma_start(out=b1_row, in_=b1_row_src)
            xhl0 = xts.tile([128, 2, K_CH, TT], f16, tag="x")
            xhl1 = xts.tile([128, 2, K_CH, TT], f16, tag="x")
            nc.sync.dma_start(out=xhl0[:, 0], in_=xt[:, 0, 0, :, :])
            nc.sync.dma_start(out=w1_sb[:, 0], in_=w1[:, 0])
            nc.sync.dma_start(out=xhl0[:, 1], in_=xt[:, 0, 1, :, :])
            nc.sync.dma_start(out=w1l_sb, in_=w1l[:, :, :])
            nc.sync.dma_start(out=xhl1[:, 0], in_=xt[:, 1, 0, :, :])
            nc.sync.dma_start(out=xhl1[:, 1], in_=xt[:, 1, 1, :, :])
            xt_tiles[0] = xhl0
            xt_tiles[1] = xhl1
            for s in range(1, N_SLAB):
                nc.sync.dma_start(out=w1_sb[:, s], in_=w1[:, s])
            prefetch_xt(2)
            nc.sync.dma_start(out=w2_sb[:, 0:8, :], in_=w2[:, 0:8, :])
            prefetch_xt(3)
            nc.sync.dma_start(out=w2_sb[:, 8:16, :], in_=w2[:, 8:16, :])
            make_identity(nc, ident)
            nc.vector.memset(ones, 1.0)

            # fp32 bias broadcast across partitions on the PE (exact:
            # 1.0 * b accumulated in fp32) while weights stream in.
            for c0, c1 in SLAB_LIM:
                pb = pl_pool.tile([TT, 512], f32, tag="pl")
                nc.tensor.matmul(pb[:, 0:c1 - c0], lhsT=ones,
                                 rhs=b1_row[:, c0:c1], start=True, stop=True)
                nc.vector.tensor_copy(b1_sb[:, c0:c1], pb[:, 0:c1 - c0])

            state = {}

            def epilogue_vec(j, lg, d1, vv, ac):
                # tree mask: V_0 = 1 at root cols; then per level
                # child1 = V_d * dec_d, child0 = V_d - child1
                nc.vector.memset(vv[:, 0:8], 1.0)
                for d in range(DEPTH):
                    ld = 8 * (1 << d)
                    c0 = 8 * ((1 << d) - 1)
                    c1 = 8 * ((1 << (d + 1)) - 1)
                    vpar = vv[:, c0:c0 + ld].rearrange("p (i t) -> p i t", t=8)
                    dpar = d1[:, c0:c0 + ld].rearrange("p (i t) -> p i t", t=8)
                    kids = vv[:, c1:c1 + 2 * ld].rearrange(
                        "p (i two t) -> p i two t", two=2, t=8)
                    nc.vector.tensor_tensor(kids[:, :, 1, :], vpar, dpar,
                                            Alu.mult)
                    nc.vector.tensor_tensor(kids[:, :, 0, :], vpar,
                                            kids[:, :, 1, :], Alu.subtract)

            def finish_mask(j, ac, vv):
                mk = mks_pool.tile([TT, NODES_PAD], f16, tag="mk")
                nc.vector.memset(mk[:, WIDTH:NODES_PAD], 0.0)
                nc.vector.tensor_tensor(mk[:, 0:1024], ac[:, 0:1024],
                                        vv[:, 0:1024], Alu.mult)
                nc.vector.tensor_tensor(mk[:, 1024:WIDTH], ac[:, 1024:WIDTH],
                                        vv[:, 1024:WIDTH], Alu.mult)
                state[j] = mk

            def gemm1_slab(s, xh, xl, xh2, lg, d1, ac):
                """Matmuls + per-slab epilogue for slab s of one tile."""
                c0, c1 = SLAB_LIM[s]
                w = c1 - c0
                p = pl_pool.tile([TT, 512], f32, tag="pl")
                if s == 0:
                    for k in range(K_CH):
                        nc.tensor.matmul(p, lhsT=xh[:, k, :],
                                         rhs=w1_sb[:, 0, k, :],
                                         start=(k == 0), stop=False)
                    for k in range(K_CH):
                        nc.tensor.matmul(p[:, 0:SH_COLS], lhsT=xl[:, k, :],
                                         rhs=w1_sb[:, 0, k, 0:SH_COLS],
                                         start=False, stop=False)
                    for k in range(K_CH):
                        nc.tensor.matmul(p[:, 0:SH_COLS], lhsT=xh2[:, k, :],
                                         rhs=w1l_sb[:, k, :],
                                         start=False, stop=(k == K_CH - 1))
                else:
                    for k in range(K_CH):
                        nc.tensor.matmul(p[:, 0:w], lhsT=xh[:, k, :],
                                         rhs=w1_sb[:, s, k, 0:w],
                                         start=(k == 0),
                                         stop=(k == K_CH - 1))
                nc.vector.tensor_tensor(lg[:, c0:c1], p[:, 0:w],
                                        b1_sb[:, c0:c1], Alu.add)
                if s == 0:
                    nc.vector.tensor_scalar(d1[:, 0:512], lg[:, 0:512], 0.0,
                                            None, Alu.is_gt)
                elif s == 1:
                    nc.vector.tensor_scalar(d1[:, 512:DEC_COLS],
                                            lg[:, 512:DEC_COLS], 0.0,
                                            None, Alu.is_gt)
                nc.scalar.activation(ac[:, c0:c1], lg[:, c0:c1], Act.Silu)

            def tile_bufs(j):
                xhl = xt_tiles.pop(j)
                xh, xl = xhl[:, 0], xhl[:, 1]
                xh2 = xh2s.tile([128, K_CH, TT], f16, tag="xh2")
                nc.vector.tensor_scalar(xh2, xh, 1.0 / LO_SCALE, None,
                                        Alu.mult)
                lg = logits_pool.tile([TT, WIDTH], f32, tag="lg")
                d1 = mask_pool.tile([TT, DEC_COLS], f16, tag="d1")
                vv = mask_pool.tile([TT, WIDTH], f16, tag="vv")
                ac = acts_pool.tile([TT, WIDTH], f16, tag="ac")
                return xh, xl, xh2, lg, d1, vv, ac

            def stage_a(j):
                if j + 1 < NTILES and j + 1 not in xt_tiles:
                    prefetch_xt(j + 1)
                xh, xl, xh2, lg, d1, vv, ac = tile_bufs(j)
                for s in range(N_SLAB):
                    gemm1_slab(s, xh, xl, xh2, lg, d1, ac)
                epilogue_vec(j, lg, d1, vv, ac)
                finish_mask(j, ac, vv)

            def stage_ab01():
                # tiles 0 and 1 slab-major: each arriving w1 slab feeds
                # 2 tiles of PE work, halving the DMA-bound startup.
                b0 = tile_bufs(0)
                b1_ = tile_bufs(1)
                for s in range(N_SLAB):
                    gemm1_slab(s, b0[0], b0[1], b0[2], b0[3], b0[4], b0[6])
                    gemm1_slab(s, b1_[0], b1_[1], b1_[2], b1_[3], b1_[4],
                               b1_[6])
                    if s == 1:
                        epilogue_vec(0, b0[3], b0[4], b0[5], b0[6])
                        epilogue_vec(1, b1_[3], b1_[4], b1_[5], b1_[6])
                finish_mask(0, b0[6], b0[5])
                finish_mask(1, b1_[6], b1_[5])

            def stage_b(j):
                mk = state.pop(j)
                at = acts_pool.tile([128, C_CH, TT], f16, tag="at")
                c = 0
                for gsz in (1, 3, 4, 4, 4):
                    pt = pt_pool.tile([128, 512], f16)
                    for i in range(gsz):
                        nc.tensor.transpose(
                            pt[:, i * 128:(i + 1) * 128],
                            mk[:, (c + i) * 128:(c + i + 1) * 128], ident)
                    nc.scalar.copy(
                        at[:, c:c + gsz, :],
                        pt[:, :gsz * 128].rearrange("p (c t) -> p c t", t=TT))
                    c += gsz
                ys = out_pool.tile([TT, DIM], f32, tag="ys")
                # c-outer so w2 chunks are consumed in arrival order
                py0 = py_pool.tile([TT, 512], f32, tag="py")
                py1 = py_pool.tile([TT, 512], f32, tag="py")
                for c in range(C_CH):
                    nc.tensor.matmul(py0, lhsT=at[:, c, :],
                                     rhs=w2_sb[:, c, 0:512],
                                     start=(c == 0), stop=(c == C_CH - 1))
                    nc.tensor.matmul(py1, lhsT=at[:, c, :],
                                     rhs=w2_sb[:, c, 512:1024],
                                     start=(c == 0), stop=(c == C_CH - 1))
                for h, py in ((0, py0), (1, py1)):
                    hs = slice(h * 512, (h + 1) * 512)
                    nc.vector.tensor_copy(ys[:, hs], py)
                    nc.sync.dma_start(out=y[j * TT:(j + 1) * TT, hs],
                                      in_=ys[:, hs])

            # pipeline: AB(0,1), A(2), B(0), A(3), B(1), ... A(7), B(5),
            # B(6), B(7)
            stage_ab01()
            for j in range(2, NTILES):
                stage_a(j)
                stage_b(j - 2)
            stage_b(NTILES - 2)
            stage_b(NTILES - 1)

    nc.finalize()
    return nc


def _get_program():
    global _PROGRAM
    if _PROGRAM is None:
        _PROGRAM = _build_program()
    return _PROGRAM


def _split_hi_lo_f16(a):
    hi = a.astype(np.float16)
    lo = (a - hi.astype(np.float32)).astype(np.float16)
    return hi, lo


def kernel(oldx, W_in, b_in, W_out):
    from concourse.bass_utils import run_bass_kernel_spmd

    oldx = np.asarray(oldx)
    W_in = np.asarray(W_in, dtype=np.float32)
    b_in = np.asarray(b_in, dtype=np.float32)
    W_out = np.asarray(W_out, dtype=np.float32)
    x = oldx.reshape(-1, DIM).astype(np.float32)          # [8192, 1024]

    # node-major column permutation: our col 8n+t  <-  ref col 255t+n
    i = np.arange(WIDTH)
    perm = 255 * (i % PAR) + (i // PAR)

    w1t = W_in[perm, :].T.astype(np.float32)              # [1024, 2040]
    w1t_hi = w1t.astype(np.float16)
    w1t_lo = ((w1t - w1t_hi.astype(np.float32)) * LO_SCALE).astype(np.float16)
    # [dim, width] -> [128, N_SLAB, K_CH, 512] with dim = k*128 + p,
    # width col = 512*slab + c (last slab zero-padded to 512)
    w1p = np.zeros((1024, N_SLAB * 512), np.float16)
    w1p[:, :WIDTH] = w1t_hi
    w1 = np.ascontiguousarray(
        w1p.reshape(K_CH, 128, N_SLAB, 512).transpose(1, 2, 0, 3))
    w1l = np.ascontiguousarray(
        w1t_lo.reshape(K_CH, 128, WIDTH).transpose(1, 0, 2)[:, :, :SH_COLS])
    b1 = np.ascontiguousarray(b_in[perm])

    w2t = np.zeros((NODES_PAD, DIM), np.float32)
    w2t[:WIDTH] = W_out.T[perm, :]
    w2 = np.ascontiguousarray(
        w2t.astype(np.float16).reshape(C_CH, 128, DIM).transpose(1, 0, 2))

    in_maps = []
    for c in range(N_CORES):
        xc = x[c * TOK_PER_CORE:(c + 1) * TOK_PER_CORE]   # [1024, 1024]
        xt_hi, xt_lo = _split_hi_lo_f16(xc.T)             # [dim, tok]
        # [dim, tok] -> [128, NTILES, K_CH, TT]; dim = k*128+p, tok = j*128+t
        xt_hi = xt_hi.reshape(K_CH, 128, NTILES, TT).transpose(1, 2, 0, 3)
        xt_lo = xt_lo.reshape(K_CH, 128, NTILES, TT).transpose(1, 2, 0, 3)
        xt = np.ascontiguousarray(np.stack([xt_hi, xt_lo], axis=2))
        in_maps.append({
            "xt": xt, "w1": w1, "w1l": w1l,
            "b1": b1, "w2": w2,
        })

    nc = _get_program()
    res = run_bass_kernel_spmd(nc, in_maps, core_ids=list(range(N_CORES)))
    out = np.concatenate([res.results[c]["y"] for c in range(N_CORES)],
                         axis=0)
    return out.reshape(oldx.shape).astype(np.float32)
